# revision 106
# baseline (speedup 1.0000x reference)
"""Trainium2 Bass kernel for BalSupMoCoNet supervised-contrastive loss.

Triangle-symmetric decomposition (sim matrix is symmetric, so each unordered
tile pair is computed once):
  N = 16640 rows = 130 tiles of 128. Core c owns row-tiles a = 8v + c
  (v = 0..16). Row a computes sim tiles (a, t) for t >= a: row sums via ACT
  accum_out (about half offloaded to DVE tensor_reduce to shave the ACT
  accumulator-read overhead), column sums via DVE accumulation of bf16 exp
  tiles into colacc, partition-reduced per 128-col tile by PE ones-matmuls.
  Per-core partial row/col sums are combined with TWO on-device AllReduces:
  wave 1 ([128, 89] - everything final by row v=11) launches ~13us before the
  loop ends and absorbs the cross-core launch skew while compute continues;
  wave 2 ([128, 41]) at the end then only pays a small aligned-AR latency.
  Each core runs the epilogue for its own rows; host sums 8 scalars.

Further head fixes vs the first working version: colacc's [128, N] memset is
gone (row v=0 tensor_copys into colacc, later rows add), and the small
memsets are emitted first so the first exp is not blocked behind DVE work.

SPMD uniformity: all cores run an identical program. Per-core variation comes
only through input data: colbuf_c = [featT[:, c*128:] | zeros(c*128)] shifts
the column space so row v's window is always local cols [8v*128, 16640).
The c*128 zero pad columns add a deterministic 128*exp(-C) per row to the
row sums (subtracted via the host-provided m_corr) and land their column
sums in local tiles >= 130-c, which the masked local->global scatter drops.
The diagonal tile is always the first 128 columns of each window; the DVE
colacc add skips it uniformly (its contribution is fully inside the row sum).
"""

import sys

import numpy as np

try:
    import concourse.bass as bass
except ImportError:
    sys.path.insert(0, "/opt/trn_rl_repo")
    import concourse.bass as bass

import concourse.bacc as bacc
import concourse.tile as tile
from concourse import mybir
from concourse.bass_utils import run_bass_kernel_spmd

# The stock walrus invocation passes --enable-ldw-opt=false; this kernel's
# row loop issues ~17 consecutive matmuls with the same stationary weights,
# so eliding the redundant LDWEIGHTS saves ~28us of PE time. Rewrite the
# flag on the compiler command line for our own compiles.
import concourse.bass_utils as _bu

if False and not getattr(_bu, "_ldw_opt_patched", False):
    # --enable-ldw-opt=true crashes walrus visitInstLdweights; keep disabled.
    _orig_run_command = _bu.run_command

    def _run_command_ldw(cmd, *a, **kw):
        cmd = ["--enable-ldw-opt=true" if c == "--enable-ldw-opt=false" else c
               for c in cmd]
        return _orig_run_command(cmd, *a, **kw)

    _bu.run_command = _run_command_ldw
    _bu._ldw_opt_patched = True

AF = mybir.ActivationFunctionType
ALU = mybir.AluOpType
F32 = mybir.dt.float32
BF16 = mybir.dt.bfloat16

W1T = 64            # wave-1 AllReduce covers global tiles [0, W1T); multiple
W1V = 8             # of 8 so rows v < W1V are exactly the wave-1 rows
MIDT = 112          # mid-scatter boundary (end of v=13), also multiple of 8
MIDV = 14
SPW = 136           # sp width: NT padded to 8*17 (pad cols Ln-neutral)
DVE_ROWSUM_EVERY = 0  # offloading rowsums to DVE measured slower at 2 and 3


class Cfg:
    def __init__(self, B=256, K=8192, ncores=8):
        self.B, self.K, self.D, self.ncores = B, K, 128, ncores
        self.T = 0.07
        self.C = 1.0 / self.T
        self.N = B + 2 * K                  # 16640
        self.NT = self.N // 128             # 130 row/col tiles
        self.VR = 17                        # row slots per core (8*17=136>=130)
        self.CHW = 1536                     # psum/ACT chunk width (3 banks)
        self.MMW = 512                      # matmul width (1 psum bank)
        self.MAXCH = 11                     # max chunks per row (v=0)


FULL = Cfg()


def _row_chunks(cfg, v):
    """Chunks (k, start, w) of row-slot v's window [8v*128, N)."""
    start0 = 8 * v * 128
    W = cfg.N - start0
    out = []
    k = 0
    while k * cfg.CHW < W:
        w = min(cfg.CHW, W - k * cfg.CHW)
        out.append((k, start0 + k * cfg.CHW, w))
        k += 1
    return out


def build_program(cfg):
    nc = bacc.Bacc("TRN2", target_bir_lowering=False, debug=False,
                   enable_asserts=True, num_devices=cfg.ncores)

    N, NT, VR, CHW, MMW = cfg.N, cfg.NT, cfg.VR, cfg.CHW, cfg.MMW
    T, C = cfg.T, cfg.C
    nco = cfg.ncores

    d_colbuf = nc.dram_tensor("colbuf", [128, N], BF16, kind="ExternalInput").ap()
    d_g01 = nc.dram_tensor("g01", [128, 2], BF16, kind="ExternalInput").ap()
    d_eii = nc.dram_tensor("m_eii", [128, NT], F32, kind="ExternalInput").ap()
    d_mw = nc.dram_tensor("m_w", [128, NT], F32, kind="ExternalInput").ap()
    d_i1l = nc.dram_tensor("m_i1l", [128, VR], F32, kind="ExternalInput").ap()
    d_rsl = nc.dram_tensor("m_rsl", [128, VR], F32, kind="ExternalInput").ap()
    d_bl = nc.dram_tensor("m_bl", [128, VR], F32, kind="ExternalInput").ap()
    d_wl = nc.dram_tensor("m_wl", [128, VR], F32, kind="ExternalInput").ap()
    d_corr = nc.dram_tensor("m_corr", [128, VR], F32, kind="ExternalInput").ap()
    d_oh = nc.dram_tensor("onehot", [128, nco], F32, kind="ExternalInput").ap()
    d_negm = nc.dram_tensor("negm", [128, 1], F32, kind="ExternalInput").ap()
    d_mw2 = nc.dram_tensor("m_w2", [16, SPW - MIDT], F32,
                           kind="ExternalInput").ap()
    d_out = nc.dram_tensor("out", [128, 1], F32, kind="ExternalOutput").ap()

    from contextlib import ExitStack
    with tile.TileContext(nc) as tc, ExitStack() as ctx:
        feat = ctx.enter_context(tc.tile_pool(name="feat", bufs=1))
        consts = ctx.enter_context(tc.tile_pool(name="consts", bufs=1))
        accs = ctx.enter_context(tc.tile_pool(name="accs", bufs=1))
        epool = ctx.enter_context(tc.tile_pool(name="epool", bufs=3))
        pspool = ctx.enter_context(tc.tile_pool(name="psum", bufs=2, space="PSUM"))
        cpspool = ctx.enter_context(tc.tile_pool(name="cpsum", bufs=2, space="PSUM"))
        dram = ctx.enter_context(tc.tile_pool(name="dram", bufs=2, space="DRAM"))

        # ---- input DMAs (colbuf split so compute starts after chunk 0) ----
        colbuf = feat.tile([128, N], BF16, tag="colbuf")
        for k in range(cfg.MAXCH):
            lo = k * CHW
            hi = min(N, lo + CHW)
            nc.sync.dma_start(out=colbuf[:, lo:hi], in_=d_colbuf[:, lo:hi])
        g01 = consts.tile([128, 2], BF16, tag="g01")
        nc.sync.dma_start(out=g01[:], in_=d_g01[:])
        m_eii = consts.tile([128, NT], F32, tag="meii")
        nc.sync.dma_start(out=m_eii[:], in_=d_eii[:])
        m_w = consts.tile([128, NT], F32, tag="mw")
        nc.sync.dma_start(out=m_w[:], in_=d_mw[:])
        m_i1l = consts.tile([128, VR], F32, tag="mi1l")
        nc.sync.dma_start(out=m_i1l[:], in_=d_i1l[:])
        m_rsl = consts.tile([128, VR], F32, tag="mrsl")
        nc.sync.dma_start(out=m_rsl[:], in_=d_rsl[:])
        m_bl = consts.tile([128, VR], F32, tag="mbl")
        nc.sync.dma_start(out=m_bl[:], in_=d_bl[:])
        m_wl = consts.tile([128, VR], F32, tag="mwl")
        nc.sync.dma_start(out=m_wl[:], in_=d_wl[:])
        m_corr = consts.tile([128, VR], F32, tag="mcorr")
        nc.sync.dma_start(out=m_corr[:], in_=d_corr[:])
        onehot = consts.tile([128, nco], F32, tag="oh")
        nc.sync.dma_start(out=onehot[:], in_=d_oh[:])
        negm = consts.tile([128, 1], F32, tag="negm")
        nc.sync.dma_start(out=negm[:], in_=d_negm[:])
        m_w2 = consts.tile([16, SPW - MIDT], F32, tag="mw2")
        nc.sync.dma_start(out=m_w2[:], in_=d_mw2[:])

        # ---- small constants / accumulators (no big colacc memset) ----
        b_negC = consts.tile([128, 1], F32, tag="negC")
        nc.vector.memset(b_negC[:], -C)
        ones_bf = consts.tile([128, 1], BF16, tag="ones")
        nc.vector.memset(ones_bf[:], 1.0)
        rowsums = accs.tile([128, VR * cfg.MAXCH], F32, tag="rowsums")
        nc.vector.memset(rowsums[:], 0.0)
        colp = accs.tile([128, NT], F32, tag="colp")        # local-layout col sums
        nc.vector.memset(colp[:], 0.0)
        # sp = globally-aligned rowparts+colparts (summed); core 0 seeds it
        # with -E_ii so the AllReduce output is S1 directly. Cols [NT, SPW)
        # are pad: 0.125 each so the 8-core sum is 1.0 and Ln gives 0.
        sp = accs.tile([128, SPW], F32, tag="sp")
        nc.vector.memset(sp[:, 0:NT], 0.0)
        nc.vector.memset(sp[:, NT:SPW], 0.125)
        rowp = accs.tile([128, VR], F32, tag="rowp")

        colacc = accs.tile([128, N], BF16, tag="colacc")

        # ---- main loop ----
        def colsum_mm(cps, i, t):
            """One PE partition-reduce of colacc tile t into cps column i."""
            nc.tensor.matmul(cps[:, i:i + 1],
                             colacc[:, t * 128:(t + 1) * 128],
                             ones_bf[:], start=True, stop=True)

        def colp_scatter(g_lo, g_hi):
            """Scatter colp into sp for GLOBAL tiles [g_lo, g_hi): core cc's
            contribution to global g comes from local tile g - cc."""
            for cc in range(nco):
                glo = max(g_lo, cc)
                ghi = min(g_hi, NT)
                if ghi <= glo:
                    continue
                nc.vector.scalar_tensor_tensor(
                    sp[:, glo:ghi], colp[:, glo - cc:ghi - cc],
                    onehot[:, cc:cc + 1],
                    sp[:, glo:ghi], op0=ALU.mult, op1=ALU.add)

        def rowsum_scatter(v_lo, v_hi, g_hi):
            """Reduce chunk sums of rows v in [v_lo, v_hi), subtract the pad
            correction, and scatter into sp rows 8v+cc (capped at g_hi)."""
            nvw = v_hi - v_lo
            nc.vector.reduce_sum(
                rowp[:, v_lo:v_hi],
                rowsums[:, v_lo * cfg.MAXCH:v_hi * cfg.MAXCH].rearrange(
                    "p (v k) -> p v k", k=cfg.MAXCH),
                axis=mybir.AxisListType.X)
            nc.vector.tensor_sub(rowp[:, v_lo:v_hi], rowp[:, v_lo:v_hi],
                                 m_corr[:, v_lo:v_hi])
            for cc in range(nco):
                nvs = len(range(8 * v_lo + cc, min(g_hi, NT), 8))
                if nvs <= 0:
                    continue
                nc.vector.scalar_tensor_tensor(
                    sp[:, 8 * v_lo + cc:min(g_hi, NT):8],
                    rowp[:, v_lo:v_lo + nvs], onehot[:, cc:cc + 1],
                    sp[:, 8 * v_lo + cc:min(g_hi, NT):8],
                    op0=ALU.mult, op1=ALU.add)

        chunk_idx = 0
        for v in range(VR):
            if v == 4:
                # core 0 seeds sp with -E_ii (negm = -1 there, 0 elsewhere)
                nc.vector.scalar_tensor_tensor(sp[:, 0:NT], m_eii[:], negm[:],
                                               sp[:, 0:NT],
                                               op0=ALU.mult, op1=ALU.add)
            lhsT = colbuf[:, 8 * v * 128:(8 * v + 1) * 128]
            chunks = _row_chunks(cfg, v)
            # tiles whose colsums finalize with this row, interleaved one or
            # two per chunk (after chunk 0) so the PE work spreads instead of
            # bunching at the row boundary and stalling ACT. Tiles beyond
            # MIDT are only needed by the final scatter: run them after the
            # loop, when PE is idle, instead of squeezing them into the last
            # rows' few chunks.
            cs_lo = 8 * v + 1 if v > 0 else 1
            cs_hi = min(8 * v + 8, NT - 1)
            cs_tiles = list(range(cs_lo, cs_hi + 1))
            cps = None
            if cs_tiles:
                cps = cpspool.tile([128, 512], F32, tag="cps")
            cs_done = 0
            nch = len(chunks)
            for k, start, w in chunks:
                if k >= 1 and nch > 1:
                    want = len(cs_tiles) * k // (nch - 1)
                    for i in range(cs_done, want):
                        colsum_mm(cps, i, cs_tiles[i])
                    cs_done = want
                ps = pspool.tile([128, CHW], F32, tag="ps")
                for j in range(-(-w // MMW)):
                    mw = min(MMW, w - j * MMW)
                    nc.tensor.matmul(ps[:, j * MMW:j * MMW + mw], lhsT,
                                     colbuf[:, start + j * MMW:start + j * MMW + mw],
                                     start=True, stop=True)
                et = epool.tile([128, CHW], BF16, tag="et")
                rs_slot = rowsums[:, v * cfg.MAXCH + k:v * cfg.MAXCH + k + 1]
                chunk_idx += 1
                if DVE_ROWSUM_EVERY and chunk_idx % DVE_ROWSUM_EVERY == 0:
                    # rowsum on DVE: saves the ~290ns ACT accumulator read
                    nc.scalar.activation(et[:, 0:w], ps[:, 0:w], AF.Exp,
                                         bias=b_negC[:], scale=1.0 / T)
                    nc.vector.reduce_sum(rs_slot, et[:, 0:w],
                                         axis=mybir.AxisListType.X)
                else:
                    nc.scalar.activation(et[:, 0:w], ps[:, 0:w], AF.Exp,
                                         bias=b_negC[:], scale=1.0 / T,
                                         accum_out=rs_slot)
                skip = 128 if k == 0 else 0
                if w > skip:
                    if v == 0:
                        nc.vector.tensor_copy(colacc[:, start + skip:start + w],
                                              et[:, skip:w])
                    else:
                        nc.vector.tensor_add(colacc[:, start + skip:start + w],
                                             colacc[:, start + skip:start + w],
                                             et[:, skip:w])
            # finish any colsum matmuls not covered inside the chunk loop,
            # then stage this row's batch into colp
            for i in range(cs_done, len(cs_tiles)):
                colsum_mm(cps, i, cs_tiles[i])
            if cs_tiles:
                nc.vector.tensor_copy(colp[:, cs_tiles[0]:cs_tiles[0] + len(cs_tiles)],
                                      cps[:, 0:len(cs_tiles)])
            if v == W1V - 1:
                # ---- wave 1: tiles [0, W1T) are fully summed into sp ----
                colp_scatter(0, W1T)
                rowsum_scatter(0, W1V, W1T)
                ar_in1 = dram.tile([128, W1T], F32)
                ar_out1 = dram.tile([128, W1T], F32, addr_space="Shared")
                nc.sync.dma_start(out=ar_in1[:], in_=sp[:, 0:W1T])
                nc.gpsimd.collective_compute(
                    "AllReduce", ALU.add,
                    replica_groups=[list(range(nco))],
                    ins=[ar_in1.opt()], outs=[ar_out1.opt()])
            if v == MIDV - 1:
                # mid scatter + mid AllReduce over tiles [W1T, MIDT): mostly
                # hides under the loop tail, so the final ReduceScatter only
                # carries [MIDT, SPW) and can start as soon as the last rows'
                # sums land
                colp_scatter(W1T, MIDT)
                rowsum_scatter(W1V, MIDV, MIDT)
                ar_inm = dram.tile([128, MIDT - W1T], F32)
                ar_outm = dram.tile([128, MIDT - W1T], F32, addr_space="Shared")
                nc.sync.dma_start(out=ar_inm[:], in_=sp[:, W1T:MIDT])
                nc.gpsimd.collective_compute(
                    "AllReduce", ALU.add,
                    replica_groups=[list(range(nco))],
                    ins=[ar_inm.opt()], outs=[ar_outm.opt()])

        # ---- wave 2: remaining tiles via ReduceScatter (cheaper than AR;
        # each rank receives partition rows [16r, 16r+16) of the summed
        # [128, SPW-W1T] block and runs the Ln epilogue on just those) ----
        colp_scatter(MIDT, NT)
        rowsum_scatter(MIDV, VR, NT)
        ar_in2 = dram.tile([128, SPW - MIDT], F32)
        ar_out2 = dram.tile([16, SPW - MIDT], F32)
        nc.sync.dma_start(out=ar_in2[:], in_=sp[:, MIDT:SPW])
        nc.gpsimd.collective_compute(
            "ReduceScatter", ALU.add,
            replica_groups=[list(range(nco))],
            ins=[ar_in2.opt()], outs=[ar_out2.opt()])

        # ---- post-AR1/AR-mid: Ln of waves 1+mid (hidden under the RS) ----
        artot1 = accs.tile([128, W1T], F32, tag="artot1")
        nc.sync.dma_start(out=artot1[:], in_=ar_out1[:])
        lg = accs.tile([128, NT], F32, tag="lg")
        nc.scalar.activation(lg[:, 0:W1T], artot1[:], AF.Ln)
        nc.vector.tensor_mul(lg[:, 0:W1T], lg[:, 0:W1T], m_w[:, 0:W1T])
        outw1 = accs.tile([128, 1], F32, tag="outw1")
        nc.vector.reduce_sum(outw1[:], lg[:, 0:W1T], axis=mybir.AxisListType.X)
        artotm = accs.tile([128, MIDT - W1T], F32, tag="artotm")
        nc.sync.dma_start(out=artotm[:], in_=ar_outm[:])
        nc.scalar.activation(lg[:, W1T:MIDT], artotm[:], AF.Ln)
        nc.vector.tensor_mul(lg[:, W1T:MIDT], lg[:, W1T:MIDT], m_w[:, W1T:MIDT])
        outwm = accs.tile([128, 1], F32, tag="outwm")
        nc.vector.reduce_sum(outwm[:], lg[:, W1T:MIDT], axis=mybir.AxisListType.X)
        nc.vector.tensor_add(outw1[:], outw1[:], outwm[:])

        # ---- dg dots + local S2 part (overlaps the collectives) ----
        dgps = pspool.tile([128, CHW], F32, tag="ps")
        for v in range(VR):
            nc.tensor.matmul(dgps[:, 2 * v:2 * v + 2],
                             colbuf[:, 8 * v * 128:(8 * v + 1) * 128],
                             g01[:], start=True, stop=True)
        dgl = accs.tile([128, 2 * VR], F32, tag="dgl")
        nc.vector.tensor_copy(dgl[:], dgps[:, 0:2 * VR])
        el = accs.tile([128, VR], F32, tag="el")
        nc.vector.tensor_sub(el[:], dgl[:, 1:2 * VR:2], dgl[:, 0:2 * VR:2])
        nc.vector.tensor_mul(el[:], el[:], m_i1l[:])
        nc.vector.tensor_add(el[:], el[:], dgl[:, 0:2 * VR:2])
        nc.vector.tensor_mul(el[:], el[:], m_rsl[:])
        nc.vector.tensor_add(el[:], el[:], m_bl[:])
        nc.vector.tensor_mul(el[:], el[:], m_wl[:])
        outv1 = accs.tile([128, 1], F32, tag="outv1")
        nc.vector.reduce_sum(outv1[:], el[:], axis=mybir.AxisListType.X)
        nc.vector.tensor_sub(outv1[:], outv1[:], outw1[:])

        # ---- post-RS epilogue: Ln over this rank's [16, SPW-MIDT] slice ----
        artot2 = accs.tile([16, SPW - MIDT], F32, tag="artot2")
        nc.sync.dma_start(out=artot2[:], in_=ar_out2[:])
        lg2 = accs.tile([16, SPW - MIDT], F32, tag="lg2")
        nc.scalar.activation(lg2[:], artot2[:], AF.Ln)
        nc.vector.tensor_mul(lg2[:], lg2[:], m_w2[:])
        w2red = accs.tile([16, 1], F32, tag="w2red")
        nc.vector.reduce_sum(w2red[:], lg2[:], axis=mybir.AxisListType.X)
        outv = accs.tile([128, 1], F32, tag="outv")
        nc.vector.tensor_copy(outv[:], outv1[:])
        nc.vector.tensor_sub(outv[0:16, :], outv[0:16, :], w2red[:])
        nc.sync.dma_start(out=d_out[:], in_=outv[:])

    nc.compile()
    return nc


def prep_in_maps(cfg, q, ba_queue, nonba_queue, targets):
    q = np.ascontiguousarray(np.asarray(q), dtype=np.float32)
    ba = np.asarray(ba_queue, dtype=np.float32)
    nb = np.asarray(nonba_queue, dtype=np.float32)
    tg = np.asarray(targets).astype(np.int64)
    B, K, N, NT, VR = cfg.B, cfg.K, cfg.N, cfg.NT, cfg.VR

    import ml_dtypes
    BF = ml_dtypes.bfloat16

    qn = q / np.clip(np.linalg.norm(q, axis=1, keepdims=True), 1e-12, None)
    featT = np.concatenate([qn.T, ba.T, nb.T], axis=1).astype(BF)   # [128, N]
    feat32 = featT.astype(np.float32)
    labels = np.concatenate([tg, np.ones(K, np.int64), np.zeros(K, np.int64)])
    g0 = feat32.astype(np.float64)[:, labels == 0].sum(axis=1)
    g1 = feat32.astype(np.float64)[:, labels == 1].sum(axis=1)
    g01 = np.stack([g0, g1], axis=1).astype(np.float32).astype(BF)  # [128, 2]
    dvec = (feat32 * feat32).sum(axis=0).astype(np.float32)         # [N]
    c1 = int(labels.sum())
    c0 = N - c1
    P = np.where(labels == 1, c1 - 1, c0 - 1).astype(np.float64)
    rs = (1.0 / (cfg.T * P)).astype(np.float32)

    def tiled(vec):
        return np.ascontiguousarray(
            np.broadcast_to(vec, (N,)).reshape(NT, 128).T.astype(np.float32))

    m_eii = tiled(np.exp(dvec.astype(np.float64) / cfg.T - cfg.C))
    i1f = labels.astype(np.float32)
    bvec = -(dvec * rs + cfg.C)
    expC = float(np.exp(-cfg.C))

    def local_rows(vec, c, fill=0.0):
        """[128, VR] layout for this core's rows a = 8v + c (dummy -> fill)."""
        out = np.full((128, VR), fill, np.float32)
        for v in range(VR):
            t = 8 * v + c
            if t < NT:
                out[:, v] = vec[t * 128:(t + 1) * 128]
        return np.ascontiguousarray(out)

    in_maps = []
    for c in range(cfg.ncores):
        colbuf = np.concatenate(
            [featT[:, c * 128:], np.zeros((128, c * 128), BF)], axis=1)
        wcol = np.where(np.arange(NT) % 8 == c, -1.0 / N, 0.0).astype(np.float32)
        m_w = np.ascontiguousarray(
            np.broadcast_to(wcol, (128, NT)).astype(np.float32))
        wl = np.full((128, VR), -1.0 / N, np.float32)
        if c >= 2:
            wl[:, VR - 1] = 0.0               # dummy slot
        corr = np.full(VR, c * 128 * expC, np.float32)
        if c >= 2:
            corr[VR - 1] = 256.0 * expC       # dummy slot: all-zero window
        m_corr = np.ascontiguousarray(
            np.broadcast_to(corr, (128, VR)).astype(np.float32))
        oh = np.zeros((128, cfg.ncores), np.float32)
        oh[:, c] = 1.0
        negm = np.full((128, 1), -1.0 if c == 0 else 0.0, np.float32)
        m_w2 = np.full((16, 136 - 112), -1.0 / N, np.float32)
        m_w2[:, NT - 112:] = 0.0              # pad tiles [130, 136)
        in_maps.append({
            "colbuf": np.ascontiguousarray(colbuf),
            "g01": np.ascontiguousarray(g01),
            "m_eii": m_eii,
            "m_w": m_w,
            "m_i1l": local_rows(i1f, c),
            "m_rsl": local_rows(rs, c),
            "m_bl": local_rows(bvec, c),
            "m_wl": np.ascontiguousarray(wl),
            "m_corr": m_corr,
            "onehot": oh,
            "negm": negm,
            "m_w2": np.ascontiguousarray(m_w2),
        })
    return in_maps


_PROGRAM = None


def get_program():
    global _PROGRAM
    if _PROGRAM is None:
        _PROGRAM = build_program(FULL)
    return _PROGRAM


def run_on_hw(in_maps, trace=False):
    nc = get_program()
    return run_bass_kernel_spmd(nc, in_maps, list(range(FULL.ncores)), trace=trace)


def kernel(q, ba_queue, nonba_queue, targets):
    in_maps = prep_in_maps(FULL, q, ba_queue, nonba_queue, targets)
    res = run_on_hw(in_maps)
    total = sum(float(r["out"].astype(np.float64).sum()) for r in res.results)
    return np.array(total, dtype=np.float32)
